# revision 1
# baseline (speedup 1.0000x reference)
"""Trainium2 Bass kernel for nn_HeatmapEncoder.

Math per (b, s, c) and per coordinate set (gaze, hand):
    g = exp(-((gx-cx)^2 + (gy-cy)^2) / (2 sigma^2))   on a 336x336 grid
    g = g / (sum(g) + eps)            (zeroed when cx+cy <= 0)
    unified = g_gaze + g_hand
    out = unified / (max(unified) + eps)

The Gaussian is separable, so each unified map is rank-2.  Each map is
generated ONCE by three K=6 bf16 matmuls (hi/lo split of each fp32
factor; the yl*xl term is dropped, rel err ~2^-16):
    rows (per set): (yh, xh), (yh, xl), (yl, xh)
Sum-normalization is folded into the x factors.  The map is drained
from PSUM by two concurrent readers: DVE (running max) and ACT
(unscaled copy to SBUF).  Peak reciprocals are computed on GPSIMD
(cross-partition max) and the final scale pass is load-balanced across
DVE / ACT before the output DMA (pipelined in groups of 2 maps).

Layout: map j = 4*b + q keeps its 6 factor rows at SBUF partitions
32*q .. 32*q+5, free block b (PE row-tiles are tied to 32-aligned
partition groups; cycling q hides LDWEIGHTS under matmuls).  Map rows
are interleaved y = 3*p + c so each map is a single contiguous DRAM
range for the output DMA.

Sharding: pure data parallel over batch B=8 across the 8 cores.
"""

import functools
from contextlib import ExitStack

import numpy as np

try:
    import concourse.bass as bass
except ImportError:  # pragma: no cover
    import sys

    sys.path.insert(0, "/opt/trn_rl_repo")
    import concourse.bass as bass

import concourse.tile as tile
from concourse import bacc, bass_isa, mybir
from concourse.bass_utils import run_bass_kernel_spmd

H = W = 336
P = 112  # partitions per y-chunk; y = 3*p + c  (c in 0..2)
NCH = 3
S_DIM, C_DIM = 8, 4
NMAPS = S_DIM * C_DIM  # 32 maps per core
NR = 2 * NMAPS  # 64 factor rows (map-major, gaze/hand interleaved)
NB = 8  # free blocks in the aligned factor layout (map j = 4*b + q)
N_CORES = 8
SIGMA = 10.0 / 336.0
EXP_SCALE = -1.0 / (2.0 * SIGMA * SIGMA)
EPS = 1e-6
GROUP = 2
# scale-pass engine per map (cycled): balance dve/act by their op costs
# (gpsimd tensor_scalar measured ~15us/op on hw - unusable)
SCALE_ENG = ("vector", "scalar", "vector", "vector", "vector", "scalar",
             "vector", "vector", "vector", "scalar", "vector", "vector",
             "vector", "scalar", "vector", "scalar")

F32 = mybir.dt.float32
BF16 = mybir.dt.bfloat16
AF = mybir.ActivationFunctionType
ALU = mybir.AluOpType
AX = mybir.AxisListType


def _emit(nc, tc, ctx, negc_in, out_t, grid_const, ystg, xstg):
    const = ctx.enter_context(tc.tile_pool(name="const", bufs=1))
    fact = ctx.enter_context(tc.tile_pool(name="fact", bufs=1))
    ffac = ctx.enter_context(tc.tile_pool(name="ffac", bufs=1))
    small = ctx.enter_context(tc.tile_pool(name="small", bufs=2))
    ustage = ctx.enter_context(tc.tile_pool(name="ustage", bufs=8))
    sstage = ctx.enter_context(tc.tile_pool(name="sstage", bufs=4))
    pmap = ctx.enter_context(tc.tile_pool(name="pmap", bufs=2, space="PSUM"))

    # ---- early ACT table preload via dummy exp on a memset tile ----
    dum = small.tile([1, 16], F32, tag="dum")
    nc.gpsimd.memset(dum[:], 0.0)
    dum2 = small.tile([1, 16], F32, tag="dum2")
    nc.scalar.activation(dum2[:], dum[:], AF.Exp, bias=0.0, scale=1.0)

    # ---- constants / inputs ----
    G = const.tile([NR, W], F32)
    nc.sync.dma_start(G[:], grid_const.ap())
    NC2 = const.tile([NR, 2], F32)
    nc.sync.dma_start(NC2[:], negc_in.ap())
    MBUF = const.tile([128, NMAPS], F32)
    nc.gpsimd.memset(MBUF[:], 0.0)

    # ---- 1-D gaussian factors, dense [64, 336] fp32 (x side first:
    # the x factors carry the normalization scale and gate the scatters) ----
    sqx = fact.tile([NR, W], F32)
    nc.scalar.activation(sqx[:], G[:], AF.Square, bias=NC2[:, 0:1], scale=1.0)
    fxv = fact.tile([NR, W], F32)
    nc.scalar.activation(fxv[:], sqx[:], AF.Exp, bias=0.0, scale=EXP_SCALE)
    sqy = fact.tile([NR, W], F32)
    nc.scalar.activation(sqy[:], G[:], AF.Square, bias=NC2[:, 1:2], scale=1.0)
    fyv = fact.tile([NR, W], F32)
    nc.scalar.activation(fyv[:], sqy[:], AF.Exp, bias=0.0, scale=EXP_SCALE)

    # x-side hi/lo split (UNSCALED - off the normalization chain, so the
    # x scatters can start early); the a-scale folds into the y side below
    xh = fact.tile([NR, W], BF16)
    nc.vector.tensor_copy(xh[:], fxv[:])
    xl = fact.tile([NR, W], BF16)
    nc.vector.tensor_sub(xl[:], fxv[:], xh[:])

    # normalization scale a = valid / (Sx*Sy + eps) folded into y factors
    sx = small.tile([NR, 1], F32, tag="sx")
    nc.vector.reduce_sum(sx[:], fxv[:], axis=AX.X)
    sy = small.tile([NR, 1], F32, tag="sy")
    nc.vector.reduce_sum(sy[:], fyv[:], axis=AX.X)
    ss = small.tile([NR, 1], F32, tag="ss")
    nc.vector.tensor_mul(ss[:], sx[:], sy[:])
    sse = small.tile([NR, 1], F32, tag="sse")
    nc.vector.tensor_scalar_add(sse[:], ss[:], EPS)
    rec = small.tile([NR, 1], F32, tag="rec")
    nc.vector.reciprocal(rec[:], sse[:])
    vs = small.tile([NR, 1], F32, tag="vs")
    nc.vector.tensor_add(vs[:], NC2[:, 0:1], NC2[:, 1:2])
    vm = small.tile([NR, 1], F32, tag="vm")  # valid: (-cx)+(-cy) < 0
    nc.vector.tensor_scalar(vm[:], vs[:], 0.0, None, op0=ALU.is_lt)
    av = small.tile([NR, 1], F32, tag="av")
    nc.vector.tensor_mul(av[:], rec[:], vm[:])
    fys = fact.tile([NR, W], F32)
    nc.vector.tensor_scalar_mul(fys[:], fyv[:], av[:, 0:1])

    # y-side hi/lo split (carries the a-scale)
    yh = fact.tile([NR, W], BF16)
    nc.vector.tensor_copy(yh[:], fys[:])
    yl = fact.tile([NR, W], BF16)
    nc.vector.tensor_sub(yl[:], fys[:], yh[:])

    # ---- bounce through DRAM into the 32-aligned 6-row layout ----
    # staging [3, 64, 336]: x first (ready early); y side (yh, yh, yl)
    nc.sync.dma_start(xstg.ap()[0], xh[:])
    nc.scalar.dma_start(xstg.ap()[1], xl[:])
    nc.scalar.dma_start(xstg.ap()[2], xh[:])
    nc.sync.dma_start(ystg.ap()[0], yh[:])
    nc.scalar.dma_start(ystg.ap()[1], yh[:])
    nc.sync.dma_start(ystg.ap()[2], yl[:])

    # per-q factor tiles so map j only depends on its own q's scatters
    FYq = [ffac.tile([128, NB, W], BF16, name=f"FY{q}", tag=f"fy{q}")
           for q in range(4)]
    FXq = [ffac.tile([128, NB, W], BF16, name=f"FX{q}", tag=f"fx{q}")
           for q in range(4)]
    ya = ystg.ap()  # [3, 64, 336]
    xa = xstg.ap()
    for q in range(4):
        for t in range(2):
            # dest rows 32q+3t+u  <-  stg[u, 8b+2q+t, x]
            nc.sync.dma_start(FYq[q][32 * q + 3 * t:32 * q + 3 * t + 3, :, :],
                              ya[:, 2 * q + t::8, :])
            nc.scalar.dma_start(FXq[q][32 * q + 3 * t:32 * q + 3 * t + 3, :, :],
                                xa[:, 2 * q + t::8, :])

    # DRAM view matching stage layout: out[m, y, x], y = 3p+c, z = 336c+x
    dview = out_t.ap().rearrange("m (p c) x -> p m (c x)", p=P)

    def map_matmuls(j, pt):
        q, b = j % 4, j // 4
        rhs = FXq[q][32 * q:32 * q + 6, b, :]
        for cix in range(NCH):
            lhsT = FYq[q][32 * q:32 * q + 6, b, cix::3]
            nc.tensor.matmul(pt[:, cix * 512:cix * 512 + W], lhsT, rhs,
                             start=True, stop=True, tile_position=(32 * q, 0))

    uts = {}
    for j0 in range(0, NMAPS, GROUP):
        # generate maps once; drain PSUM with DVE max + ACT unscaled copy
        for j in range(j0, j0 + GROUP):
            pt = pmap.tile([P, NCH * 512], F32, tag="pmap")
            map_matmuls(j, pt)
            pview = pt[:].rearrange("p (c z) -> p c z", c=NCH)[:, :, 0:W]
            # ACT copy is the SOLE psum reader (frees the slot); the max
            # reduce reads the SBUF copy, off the PSUM critical path
            ut = ustage.tile([P, NCH * W], F32, tag="ust")
            uview = ut[:].rearrange("p (c x) -> p c x", c=NCH)
            nc.scalar.activation(uview, pview, AF.Copy, bias=0.0, scale=1.0)
            nc.vector.reduce_max(MBUF[0:P, j:j + 1], ut[:], axis=AX.X)
            uts[j] = ut

        # peak chain on GPSIMD + DVE: rg = 1/(colmax(MBUF cols) + eps)
        par = small.tile([128, GROUP], F32, tag="par")
        nc.gpsimd.partition_all_reduce(par[:], MBUF[:, j0:j0 + GROUP],
                                       channels=128,
                                       reduce_op=bass_isa.ReduceOp.max)
        pke = small.tile([P, GROUP], F32, tag="pke")
        nc.vector.tensor_scalar_add(pke[:], par[0:P, :], EPS)
        rg = small.tile([P, GROUP], F32, tag="rg")
        nc.vector.reciprocal(rg[:], pke[:])

        # scale pass (load-balanced) + DMA out per 2 maps
        for j0p in range(j0, j0 + GROUP, 2):
            st = sstage.tile([P, 2, NCH * W], F32, tag="sst")
            for j in (j0p, j0p + 1):
                gi, si = j - j0, j - j0p
                ename = SCALE_ENG[j % len(SCALE_ENG)]
                if ename == "scalar":
                    nc.scalar.mul(st[:, si, :], uts[j][:], rg[:, gi:gi + 1])
                else:
                    getattr(nc, ename).tensor_scalar_mul(
                        st[:, si, :], uts[j][:], rg[:, gi:gi + 1])
                del uts[j]
            nc.sync.dma_start(dview[:, j0p:j0p + 2, :], st[:])


@functools.lru_cache(maxsize=1)
def _build():
    nc = bacc.Bacc("TRN2", target_bir_lowering=False, debug=False)
    negc_in = nc.dram_tensor("negc", [NR, 2], F32, kind="ExternalInput")
    out_t = nc.dram_tensor("out", [NMAPS, H, W], F32, kind="ExternalOutput")

    grid = (np.arange(W, dtype=np.float64) / (W - 1)).astype(np.float32)
    grid_const = nc.inline_tensor(np.tile(grid, (NR, 1)), name="gridc")

    ystg = nc.dram_tensor("ystg", [3, NR, W], BF16)
    xstg = nc.dram_tensor("xstg", [3, NR, W], BF16)

    with tile.TileContext(nc) as tc, ExitStack() as ctx:
        _emit(nc, tc, ctx, negc_in, out_t, grid_const, ystg, xstg)
    nc.compile()
    return nc


def _in_map_for(gaze, hand, b):
    cg = np.asarray(gaze[b], dtype=np.float32).reshape(NMAPS, 2)
    ch = np.asarray(hand[b], dtype=np.float32).reshape(NMAPS, 2)
    inter = np.stack([cg, ch], axis=1).reshape(NR, 2)  # row 2*j + t
    return {"negc": np.ascontiguousarray(-inter)}


def kernel(gaze_coords, hand_coords, _trace=False, **trace_kwargs):
    gaze_coords = np.asarray(gaze_coords, dtype=np.float32)
    hand_coords = np.asarray(hand_coords, dtype=np.float32)
    B = gaze_coords.shape[0]
    assert B == N_CORES, f"expected batch {N_CORES}, got {B}"
    nc = _build()
    in_maps = [_in_map_for(gaze_coords, hand_coords, b) for b in range(B)]
    res = run_bass_kernel_spmd(nc, in_maps, list(range(N_CORES)),
                               trace=_trace, **trace_kwargs)
    out = np.stack(
        [res.results[i]["out"].reshape(S_DIM, C_DIM, H, W) for i in range(B)],
        axis=0,
    ).astype(np.float32)
    if _trace:
        return out, res
    return out



# revision 5
# speedup vs baseline: 1.1644x; 1.1644x over previous
"""Trainium2 Bass kernel for nn_HeatmapEncoder.

Math per (b, s, c):
    g_t = exp(-((gx-cx_t)^2 + (gy-cy_t)^2) / (2 sigma^2)),  t in {gaze, hand}
    g_t = g_t / (sum(g_t) + eps)        (zeroed when cx+cy <= 0)
    unified = g_gaze + g_hand
    out = unified / (max(unified) + eps)

Each normalized Gaussian is separable, so unified is rank-2 and each map
is generated by ONE K=2 bf16 matmul per 112-row chunk (y factors carry
the per-set sum-normalization scale).  The peak max(unified) is NOT
computed from the generated map: all critical points of a sum of two
isotropic Gaussians lie on the line through the two centers, so the peak
is evaluated on 16 candidate offsets (sigma/8 apart) from each center
toward the other, using only the 1-D factor sums and the center distance
(host-precomputed).  Worst-case peak error ~0.25% and bf16 factor error
~0.4%, well inside the 2e-2 gate.

The PSUM drain is then a single fused pass: out_bf16 = psum * (1/(peak+
eps)) on DVE/ACT (alternating), followed by a bf16 output DMA (half the
fp32 bytes; host upcasts to fp32).

Layout: rows are t-major (gaze rows 0..31, hand rows 32..63) so the
partner-amplitude swap and pair-max are plain partition-shifted AP views.
Map j = 4b + q keeps its factor pair at SBUF partitions 32q, 32q+1, free
block b (PE row tiles are 32-aligned; cycling q hides LDWEIGHTS).  Map
rows are interleaved y = 3p + c so each map is one contiguous DRAM range.

Sharding: pure data parallel over batch B=8 across the 8 cores.
"""

import functools
from contextlib import ExitStack

import numpy as np

try:
    import concourse.bass as bass
except ImportError:  # pragma: no cover
    import sys

    sys.path.insert(0, "/opt/trn_rl_repo")
    import concourse.bass as bass

import concourse.tile as tile
from concourse import bacc, mybir
from concourse.bass_utils import run_bass_kernel_spmd

H = W = 336
P = 112  # partitions per y-chunk; y = 3*p + c  (c in 0..2)
NCH = 3
S_DIM, C_DIM = 8, 4
NMAPS = S_DIM * C_DIM  # 32 maps per core
NR = 2 * NMAPS  # 64 factor rows, t-major: row = 32*t + j
N_CORES = 8
SIGMA = 10.0 / 336.0
EXP_SCALE = -1.0 / (2.0 * SIGMA * SIGMA)
EPS = 1e-6
NCAND = 16  # candidate peak offsets k*sigma/8 toward the partner center
CSTEP = SIGMA / 8.0

F32 = mybir.dt.float32
BF16 = mybir.dt.bfloat16
AF = mybir.ActivationFunctionType
ALU = mybir.AluOpType
AX = mybir.AxisListType


def _emit(nc, tc, ctx, negcd_in, out_t, const_in, stg, rgd, avd):
    const = ctx.enter_context(tc.tile_pool(name="const", bufs=1))
    fact = ctx.enter_context(tc.tile_pool(name="fact", bufs=1))
    ffac = ctx.enter_context(tc.tile_pool(name="ffac", bufs=1))
    small = ctx.enter_context(tc.tile_pool(name="small", bufs=2))
    sstage = ctx.enter_context(tc.tile_pool(name="sstage", bufs=4))
    pmap = ctx.enter_context(tc.tile_pool(name="pmap", bufs=2, space="PSUM"))

    # ---- early ACT table preload via dummy exp on a memset tile ----
    dum = small.tile([1, 16], F32, tag="dum")
    nc.gpsimd.memset(dum[:], 0.0)
    dum2 = small.tile([1, 16], F32, tag="dum2")
    nc.scalar.activation(dum2[:], dum[:], AF.Exp, bias=0.0, scale=1.0)

    # ---- constants / inputs ----
    # CG packs [grid | E1 | tk] row-identically; NCD packs [-cx, -cy, -d]
    CG = const.tile([NR, W + 2 * NCAND], F32)
    nc.sync.dma_start(CG[:], const_in.ap())
    G = CG[:, 0:W]
    E1c = CG[:, W:W + NCAND]
    TKc = CG[:, W + NCAND:W + 2 * NCAND]
    NCD = const.tile([NR, 3], F32)
    nc.gpsimd.dma_start(NCD[:], negcd_in.ap())

    # ---- 1-D gaussian factors, dense [64, 336] fp32 (x side first) ----
    sqx = fact.tile([NR, W], F32)
    nc.scalar.activation(sqx[:], G, AF.Square, bias=NCD[:, 0:1], scale=1.0)
    fxv = fact.tile([NR, W], F32)
    nc.scalar.activation(fxv[:], sqx[:], AF.Exp, bias=0.0, scale=EXP_SCALE)
    xbf = fact.tile([NR, W], BF16)
    nc.vector.tensor_copy(xbf[:], fxv[:])
    nc.sync.dma_start(stg.ap()[0], xbf[:])
    sx = small.tile([NR, 1], F32, tag="sx")
    nc.vector.reduce_sum(sx[:], fxv[:], axis=AX.X)

    sqy = fact.tile([NR, W], F32)
    nc.scalar.activation(sqy[:], G, AF.Square, bias=NCD[:, 1:2], scale=1.0)
    fyv = fact.tile([NR, W], F32)
    nc.scalar.activation(fyv[:], sqy[:], AF.Exp, bias=0.0, scale=EXP_SCALE)
    sy = small.tile([NR, 1], F32, tag="sy")
    nc.vector.reduce_sum(sy[:], fyv[:], axis=AX.X)

    # per-set amplitude a = valid / (Sx*Sy + eps), folded into y factors
    ss = small.tile([NR, 1], F32, tag="ss")
    nc.vector.tensor_mul(ss[:], sx[:], sy[:])
    sse = small.tile([NR, 1], F32, tag="sse")
    nc.vector.tensor_scalar_add(sse[:], ss[:], EPS)
    rec = small.tile([NR, 1], F32, tag="rec")
    nc.vector.reciprocal(rec[:], sse[:])
    vs = small.tile([NR, 1], F32, tag="vs")
    nc.vector.tensor_add(vs[:], NCD[:, 0:1], NCD[:, 1:2])
    vm = small.tile([NR, 1], F32, tag="vm")  # valid: (-cx)+(-cy) < 0
    nc.vector.tensor_scalar(vm[:], vs[:], 0.0, None, op0=ALU.is_lt)
    av = small.tile([NR, 1], F32, tag="av")
    nc.vector.tensor_mul(av[:], rec[:], vm[:])
    fya = fact.tile([NR, W], F32)
    nc.vector.tensor_scalar_mul(fya[:], fyv[:], av[:, 0:1])
    ybf = fact.tile([NR, W], BF16)
    nc.vector.tensor_copy(ybf[:], fya[:])
    nc.gpsimd.dma_start(stg.ap()[1], ybf[:])

    # ---- candidate peak: u(t_k) = a*E1[k] + a_partner*exp(S*(t_k-d)^2),
    # candidates from each center toward the other; true max over grid is
    # within ~0.25% of the best candidate.  Evaluated map-major [32, .]
    # (engines need equal base partitions), so bounce the amplitudes
    # through DRAM into [map, (own, partner)] ----
    nc.sync.dma_start(avd.ap(), av[:])
    avm = small.tile([NMAPS, 2], F32, tag="avm")
    nc.gpsimd.dma_start(avm[:], avd.ap().rearrange("(t j) -> j t", t=2))
    sq2 = small.tile([NMAPS, NCAND], F32, tag="sq2")
    nc.scalar.activation(sq2[:], TKc[0:NMAPS, :], AF.Square,
                         bias=NCD[0:NMAPS, 2:3], scale=1.0)
    e2 = small.tile([NMAPS, NCAND], F32, tag="e2")
    nc.scalar.activation(e2[:], sq2[:], AF.Exp, bias=0.0, scale=EXP_SCALE)
    uu = small.tile([NMAPS, 2 * NCAND], F32, tag="uu")
    nc.vector.tensor_scalar_mul(uu[:, 0:NCAND], E1c[0:NMAPS, :], avm[:, 0:1])
    nc.vector.tensor_scalar_mul(uu[:, NCAND:], E1c[0:NMAPS, :], avm[:, 1:2])
    up = small.tile([NMAPS, 2 * NCAND], F32, tag="up")
    nc.vector.tensor_scalar_mul(up[:, 0:NCAND], e2[:], avm[:, 1:2])
    nc.vector.tensor_scalar_mul(up[:, NCAND:], e2[:], avm[:, 0:1])
    ub = small.tile([NMAPS, 2 * NCAND], F32, tag="ub")
    nc.vector.tensor_add(ub[:], uu[:], up[:])
    pmx = small.tile([NMAPS, 1], F32, tag="pmx")
    nc.vector.reduce_max(pmx[:], ub[:], axis=AX.X)
    pke = small.tile([NMAPS, 1], F32, tag="pke")
    nc.vector.tensor_scalar_add(pke[:], pmx[:], EPS)
    rg = small.tile([NMAPS, 1], F32, tag="rg")
    nc.vector.reciprocal(rg[:], pke[:])
    # bounce through DRAM to transpose+broadcast into [112, 32]
    nc.sync.dma_start(rgd.ap(), rg[:])
    rgB = const.tile([P, NMAPS], F32)
    nc.gpsimd.dma_start(rgB[:],
                        rgd.ap().unsqueeze(0).broadcast_to((P, NMAPS)))

    # ---- scatter factor pairs into the 32-aligned K=2 layout ----
    # dest partitions 32q+u, free block b  <-  stage row 32u + 4b + q
    FXT = ffac.tile([128, S_DIM, W], BF16, name="FXT", tag="fxt")
    FYT = ffac.tile([128, S_DIM, W], BF16, name="FYT", tag="fyt")
    for u in range(2):
        src_x = stg.ap()[0].rearrange("(u b q) x -> u q b x", u=2, q=4)[u]
        src_y = stg.ap()[1].rearrange("(u b q) x -> u q b x", u=2, q=4)[u]
        dst_x = FXT[:].rearrange("(q u) b x -> u q b x", q=4)[u]
        dst_y = FYT[:].rearrange("(q u) b x -> u q b x", q=4)[u]
        eng = nc.sync if u == 0 else nc.gpsimd
        eng.dma_start(dst_x, src_x)
        eng.dma_start(dst_y, src_y)

    # DRAM view matching stage layout: out[m, y, x], y = 3p+c, z = 336c+x
    dview = out_t.ap().rearrange("m (p c) x -> p m (c x)", p=P)

    for j0 in range(0, NMAPS, 2):
        st = sstage.tile([P, 2, NCH * W], BF16, tag="sst")
        for j in (j0, j0 + 1):
            q, b = j % 4, j // 4
            pt = pmap.tile([P, NCH * 512], F32, tag="pmap")
            rhs = FXT[32 * q:32 * q + 2, b, :]
            for cix in range(NCH):
                lhsT = FYT[32 * q:32 * q + 2, b, cix::3]
                nc.tensor.matmul(pt[:, cix * 512:cix * 512 + W], lhsT, rhs,
                                 start=True, stop=True,
                                 tile_position=(32 * q, 0))
            pview = pt[:].rearrange("p (c z) -> p c z", c=NCH)[:, :, 0:W]
            sview = st[:, j - j0, :].rearrange("p (c x) -> p c x", c=NCH)
            if j % 2 == 0:
                nc.scalar.mul(sview, pview, rgB[:, j:j + 1])
            else:
                nc.vector.tensor_scalar_mul(sview, pview, rgB[:, j:j + 1])
        eng = nc.sync if (j0 // 2) % 2 == 0 else nc.gpsimd
        eng.dma_start(dview[:, j0:j0 + 2, :], st[:])


@functools.lru_cache(maxsize=1)
def _build():
    nc = bacc.Bacc("TRN2", target_bir_lowering=False, debug=False)
    negcd_in = nc.dram_tensor("negcd", [NR, 3], F32, kind="ExternalInput")
    out_t = nc.dram_tensor("out", [NMAPS, H, W], BF16, kind="ExternalOutput")

    grid = (np.arange(W, dtype=np.float64) / (W - 1)).astype(np.float32)
    tk = (np.arange(NCAND, dtype=np.float64) * CSTEP).astype(np.float32)
    e1 = np.exp(-(tk.astype(np.float64) ** 2) / (2.0 * SIGMA ** 2)).astype(
        np.float32)
    row = np.concatenate([grid, e1, tk])
    const_in = nc.inline_tensor(np.tile(row, (NR, 1)), name="gridc")

    stg = nc.dram_tensor("stg", [2, NR, W], BF16)
    rgd = nc.dram_tensor("rgd", [NMAPS], F32)
    avd = nc.dram_tensor("avd", [NR], F32)

    with tile.TileContext(nc) as tc, ExitStack() as ctx:
        _emit(nc, tc, ctx, negcd_in, out_t, const_in, stg, rgd, avd)
    nc.compile()
    return nc


def _in_map_for(gaze, hand, b):
    cg = np.asarray(gaze[b], dtype=np.float32).reshape(NMAPS, 2)
    ch = np.asarray(hand[b], dtype=np.float32).reshape(NMAPS, 2)
    d = np.sqrt(((cg - ch) ** 2).sum(axis=1))
    negc = -np.concatenate([cg, ch], axis=0)  # t-major rows
    negd = -np.concatenate([d, d])[:, None]
    return {"negcd": np.ascontiguousarray(
        np.concatenate([negc, negd], axis=1).astype(np.float32))}


def kernel(gaze_coords, hand_coords, _trace=False, **trace_kwargs):
    gaze_coords = np.asarray(gaze_coords, dtype=np.float32)
    hand_coords = np.asarray(hand_coords, dtype=np.float32)
    B = gaze_coords.shape[0]
    assert B == N_CORES, f"expected batch {N_CORES}, got {B}"
    nc = _build()
    in_maps = [_in_map_for(gaze_coords, hand_coords, b) for b in range(B)]
    res = run_bass_kernel_spmd(nc, in_maps, list(range(N_CORES)),
                               trace=_trace, **trace_kwargs)
    out = np.stack(
        [np.asarray(res.results[i]["out"]).astype(np.float32).reshape(
            S_DIM, C_DIM, H, W) for i in range(B)],
        axis=0,
    )
    if _trace:
        return out, res
    return out


# revision 7
# speedup vs baseline: 1.7232x; 1.4799x over previous
"""Trainium2 Bass kernel for nn_HeatmapEncoder.

Math per (b, s, c):
    g_t = exp(-((gx-cx_t)^2 + (gy-cy_t)^2) / (2 sigma^2)),  t in {gaze, hand}
    g_t = g_t / (sum(g_t) + eps)        (zeroed when cx+cy <= 0)
    unified = g_gaze + g_hand
    out = unified / (max(unified) + eps)

Each normalized Gaussian is separable, so unified is rank-2 and each map
is generated by ONE K=2 bf16 matmul per 112-row chunk (y factors carry
the per-set sum-normalization amplitude a).  The peak max(unified) is
NOT computed from the generated map: all critical points of a sum of two
isotropic Gaussians lie on the line through the two centers, so the peak
is evaluated at 16 candidate offsets (sigma/8 apart) from each center
toward the other, needing only the 1-D factor sums and the center
distance (host-precomputed).  Peak error ~0.25%, bf16 factors ~0.4% --
well inside the 2e-2 gate.

Cross-partition data movement in the peak chain stays on-chip via tiny
PE matmuls: a [64,64] permutation matmul swaps partner amplitudes and
pair values (max(a,b) = (a+b+|a-b|)/2), and a transpose + ones-broadcast
matmul turns the [32] per-map reciprocals into the [112, 32] scale tile
the drains read.  The PSUM drain is one fused pass per 512-col bank:
out_bf16 = psum * (1/(peak+eps)) on DVE/ACT (balanced), followed by a
bf16 output DMA per map pair (host upcasts to fp32).

Layout: factor rows are t-major (gaze rows 0..31, hand rows 32..63).
Map j = 4b + q keeps its factor pair at SBUF partitions 32q, 32q+1, free
block b (PE row tiles are 32-aligned; cycling q hides LDWEIGHTS).  Map
rows are interleaved y = 3p + c so each map is one contiguous DRAM range.

Sharding: pure data parallel over batch B=8 across the 8 cores.
"""

import functools
from contextlib import ExitStack

import numpy as np

try:
    import concourse.bass as bass
except ImportError:  # pragma: no cover
    import sys

    sys.path.insert(0, "/opt/trn_rl_repo")
    import concourse.bass as bass

import concourse.tile as tile
from concourse import bacc, mybir
from concourse.bass_utils import run_bass_kernel_spmd

H = W = 336
P = 112  # partitions per y-chunk; y = 3*p + c  (c in 0..2)
NCH = 3
S_DIM, C_DIM = 8, 4
NMAPS = S_DIM * C_DIM  # 32 maps per core
NR = 2 * NMAPS  # 64 factor rows, t-major: row = 32*t + j
N_CORES = 8
SIGMA = 10.0 / 336.0
EXP_SCALE = -1.0 / (2.0 * SIGMA * SIGMA)
EPS = 1e-6
NCAND = 16  # candidate peak offsets k*sigma/8 toward the partner center
CSTEP = SIGMA / 8.0
N_ACT_DRAIN = 51  # of 96 chunk drains on ACT (rest on DVE), ~balanced

F32 = mybir.dt.float32
BF16 = mybir.dt.bfloat16
AF = mybir.ActivationFunctionType
ALU = mybir.AluOpType
AX = mybir.AxisListType


def _drain_engines():
    engs, acc = [], 0
    for _ in range(3 * NMAPS):
        acc += N_ACT_DRAIN
        if acc >= 3 * NMAPS:
            acc -= 3 * NMAPS
            engs.append("scalar")
        else:
            engs.append("vector")
    return engs


def _emit(nc, tc, ctx, negcd_in, out_t, const_in, aux_in, stg):
    const = ctx.enter_context(tc.tile_pool(name="const", bufs=1))
    fact = ctx.enter_context(tc.tile_pool(name="fact", bufs=1))
    ffac = ctx.enter_context(tc.tile_pool(name="ffac", bufs=1))
    small = ctx.enter_context(tc.tile_pool(name="small", bufs=2))
    sstage = ctx.enter_context(tc.tile_pool(name="sstage", bufs=4))
    pchunk = ctx.enter_context(tc.tile_pool(name="pchunk", bufs=7,
                                            space="PSUM"))
    paux = ctx.enter_context(tc.tile_pool(name="paux", bufs=1, space="PSUM"))

    # ---- early ACT table preload via dummy exp on a memset tile ----
    dum = small.tile([1, 16], F32, tag="dum")
    nc.gpsimd.memset(dum[:], 0.0)
    dum2 = small.tile([1, 16], F32, tag="dum2")
    nc.scalar.activation(dum2[:], dum[:], AF.Exp, bias=0.0, scale=1.0)

    # ---- constants / inputs ----
    # CG packs [grid | E1 | tk] row-identically; NCD packs [-cx, -cy, -d];
    # AUX packs [Pswap | I32 | ones-row] for the PE permutation tricks
    CG = const.tile([NR, W + 2 * NCAND], F32)
    nc.sync.dma_start(CG[:], const_in.ap())
    G = CG[:, 0:W]
    E1c = CG[:, W:W + NCAND]
    TKc = CG[:, W + NCAND:W + 2 * NCAND]
    NCD = const.tile([NR, 3], F32)
    nc.gpsimd.dma_start(NCD[:], negcd_in.ap())
    AUX = const.tile([NR, NR + NMAPS + P], F32)
    nc.scalar.dma_start(AUX[:], aux_in.ap())
    PswapC = AUX[:, 0:NR]
    ID32c = AUX[0:NMAPS, NR:NR + NMAPS]
    onesC = AUX[0:1, NR + NMAPS:NR + NMAPS + P]

    # ---- 1-D gaussian factors, straight to bf16 (x side first) ----
    sqx = fact.tile([NR, W], F32)
    nc.scalar.activation(sqx[:], G, AF.Square, bias=NCD[:, 0:1], scale=1.0)
    xbf = fact.tile([NR, W], BF16)
    nc.scalar.activation(xbf[:], sqx[:], AF.Exp, bias=0.0, scale=EXP_SCALE)
    nc.sync.dma_start(stg.ap()[0], xbf[:])
    sx = small.tile([NR, 1], F32, tag="sx")
    nc.vector.reduce_sum(sx[:], xbf[:], axis=AX.X)

    sqy = fact.tile([NR, W], F32)
    nc.scalar.activation(sqy[:], G, AF.Square, bias=NCD[:, 1:2], scale=1.0)
    ybv = fact.tile([NR, W], BF16)
    nc.scalar.activation(ybv[:], sqy[:], AF.Exp, bias=0.0, scale=EXP_SCALE)
    sy = small.tile([NR, 1], F32, tag="sy")
    nc.vector.reduce_sum(sy[:], ybv[:], axis=AX.X)

    # candidate partner-distance exponentials (ACT, off critical path)
    sq2 = small.tile([NR, NCAND], F32, tag="sq2")
    nc.scalar.activation(sq2[:], TKc, AF.Square, bias=NCD[:, 2:3], scale=1.0)
    e2 = small.tile([NR, NCAND], F32, tag="e2")
    nc.scalar.activation(e2[:], sq2[:], AF.Exp, bias=0.0, scale=EXP_SCALE)

    # per-set amplitude a = valid / (Sx*Sy + eps), folded into y factors
    ss = small.tile([NR, 1], F32, tag="ss")
    nc.vector.tensor_mul(ss[:], sx[:], sy[:])
    sse = small.tile([NR, 1], F32, tag="sse")
    nc.vector.tensor_scalar_add(sse[:], ss[:], EPS)
    rec = small.tile([NR, 1], F32, tag="rec")
    nc.vector.reciprocal(rec[:], sse[:])
    vs = small.tile([NR, 1], F32, tag="vs")
    nc.vector.tensor_add(vs[:], NCD[:, 0:1], NCD[:, 1:2])
    vm = small.tile([NR, 1], F32, tag="vm")  # valid: (-cx)+(-cy) < 0
    nc.vector.tensor_scalar(vm[:], vs[:], 0.0, None, op0=ALU.is_lt)
    av = small.tile([NR, 1], F32, tag="av")
    nc.vector.tensor_mul(av[:], rec[:], vm[:])
    fya = fact.tile([NR, W], BF16)
    nc.vector.tensor_scalar_mul(fya[:], ybv[:], av[:, 0:1])
    nc.gpsimd.dma_start(stg.ap()[1], fya[:])

    # ---- scatter factor pairs into the 32-aligned K=2 layout ----
    # dest partitions 32q+u, free block b  <-  stage row 32u + 4b + q
    FXT = ffac.tile([128, S_DIM, W], BF16, name="FXT", tag="fxt")
    FYT = ffac.tile([128, S_DIM, W], BF16, name="FYT", tag="fyt")
    for u in range(2):
        src_x = stg.ap()[0].rearrange("(u b q) x -> u q b x", u=2, q=4)[u]
        src_y = stg.ap()[1].rearrange("(u b q) x -> u q b x", u=2, q=4)[u]
        dst_x = FXT[:].rearrange("(q u) b x -> u q b x", q=4)[u]
        dst_y = FYT[:].rearrange("(q u) b x -> u q b x", q=4)[u]
        eng = nc.sync if u == 0 else nc.gpsimd
        eng.dma_start(dst_x, src_x)
        eng.dma_start(dst_y, src_y)

    # ---- candidate peak: u(t_k) = a*E1[k] + a_partner*exp(S*(t_k-d)^2).
    # Partner amplitudes via PE permutation matmul (rows 2-partition-swap);
    # pair-max via max(a,b) = (a+b+|a-b|)/2 on a second permuted matmul ----
    avpP = paux.tile([NR, 1], F32, tag="aux")
    nc.tensor.matmul(avpP[:], PswapC, av[:], start=True, stop=True,
                     tile_position=(0, 0))
    avp = small.tile([NR, 1], F32, tag="avp")
    nc.vector.tensor_copy(avp[:], avpP[:])
    uu = small.tile([NR, NCAND], F32, tag="uu")
    nc.vector.tensor_scalar_mul(uu[:], E1c, av[:, 0:1])
    up = small.tile([NR, NCAND], F32, tag="up")
    nc.vector.tensor_scalar_mul(up[:], e2[:], avp[:, 0:1])
    ub = small.tile([NR, NCAND], F32, tag="ub")
    nc.vector.tensor_add(ub[:], uu[:], up[:])
    pm = small.tile([NR, 1], F32, tag="pm")
    nc.vector.reduce_max(pm[:], ub[:], axis=AX.X)
    pswP = paux.tile([NR, 1], F32, tag="aux")
    nc.tensor.matmul(pswP[:], PswapC, pm[:], start=True, stop=True,
                     tile_position=(0, 0))
    sm = small.tile([NR, 1], F32, tag="sm")
    nc.vector.tensor_add(sm[:], pm[:], pswP[:])
    df = small.tile([NR, 1], F32, tag="df")
    nc.vector.tensor_sub(df[:], pm[:], pswP[:])
    ng = small.tile([NR, 1], F32, tag="ng")
    nc.vector.tensor_scalar_mul(ng[:], df[:], -1.0)
    ad = small.tile([NR, 1], F32, tag="ad")
    nc.vector.tensor_max(ad[:], df[:], ng[:])
    mx = small.tile([NR, 1], F32, tag="mx")
    nc.vector.tensor_add(mx[:], sm[:], ad[:])
    pke = small.tile([NR, 1], F32, tag="pke")  # 0.5*(s+|d|) + eps
    nc.vector.tensor_scalar(pke[:], mx[:], 0.5, EPS, op0=ALU.mult,
                            op1=ALU.add)
    rg = small.tile([NR, 1], F32, tag="rg")
    nc.vector.reciprocal(rg[:], pke[:])
    # transpose rows 0..31 into a free-dim row, then ones-broadcast to [112]
    rgTP = paux.tile([1, NMAPS], F32, tag="aux")
    nc.tensor.matmul(rgTP[:], rg[0:NMAPS, 0:1], ID32c, start=True, stop=True,
                     is_transpose=True, tile_position=(0, 0))
    rgT = small.tile([1, NMAPS], F32, tag="rgT")
    nc.vector.tensor_copy(rgT[:], rgTP[:])
    rgBP = paux.tile([P, NMAPS], F32, tag="aux")
    nc.tensor.matmul(rgBP[:], onesC, rgT[:], start=True, stop=True,
                     tile_position=(0, 0))
    rgB = const.tile([P, NMAPS], F32)
    nc.vector.tensor_copy(rgB[:], rgBP[:])

    # DRAM view matching stage layout: out[m, y, x], y = 3p+c, z = 336c+x
    dview = out_t.ap().rearrange("m (p c) x -> p m (c x)", p=P)

    engs = _drain_engines()
    for j0 in range(0, NMAPS, 2):
        st = sstage.tile([P, 2, NCH * W], BF16, tag="sst")
        for j in (j0, j0 + 1):
            q, b = j % 4, j // 4
            rhs = FXT[32 * q:32 * q + 2, b, :]
            for cix in range(NCH):
                ptc = pchunk.tile([P, 512], F32, tag="pc")
                lhsT = FYT[32 * q:32 * q + 2, b, cix::3]
                nc.tensor.matmul(ptc[:, 0:W], lhsT, rhs, start=True,
                                 stop=True, tile_position=(32 * q, 0))
                sview = st[:, j - j0, cix * W:(cix + 1) * W]
                if engs[3 * j + cix] == "scalar":
                    nc.scalar.mul(sview, ptc[:, 0:W], rgB[:, j:j + 1])
                else:
                    nc.vector.tensor_scalar_mul(sview, ptc[:, 0:W],
                                                rgB[:, j:j + 1])
        eng = nc.sync if (j0 // 2) % 2 == 0 else nc.gpsimd
        eng.dma_start(dview[:, j0:j0 + 2, :], st[:])


@functools.lru_cache(maxsize=1)
def _build():
    nc = bacc.Bacc("TRN2", target_bir_lowering=False, debug=False)
    negcd_in = nc.dram_tensor("negcd", [NR, 3], F32, kind="ExternalInput")
    out_t = nc.dram_tensor("out", [NMAPS, H, W], BF16, kind="ExternalOutput")

    grid = (np.arange(W, dtype=np.float64) / (W - 1)).astype(np.float32)
    tk = (np.arange(NCAND, dtype=np.float64) * CSTEP).astype(np.float32)
    e1 = np.exp(-(tk.astype(np.float64) ** 2) / (2.0 * SIGMA ** 2)).astype(
        np.float32)
    row = np.concatenate([grid, e1, tk])
    const_in = nc.inline_tensor(np.tile(row, (NR, 1)), name="gridc")

    aux = np.zeros((NR, NR + NMAPS + P), dtype=np.float32)
    for k in range(NR):  # Pswap: out[m] = in[(m+32)%64]
        aux[k, (k + NMAPS) % NR] = 1.0
    aux[0:NMAPS, NR:NR + NMAPS] = np.eye(NMAPS, dtype=np.float32)
    aux[0, NR + NMAPS:NR + NMAPS + P] = 1.0
    aux_in = nc.inline_tensor(aux, name="auxc")

    stg = nc.dram_tensor("stg", [2, NR, W], BF16)

    with tile.TileContext(nc) as tc, ExitStack() as ctx:
        _emit(nc, tc, ctx, negcd_in, out_t, const_in, aux_in, stg)
    nc.compile()
    return nc


def _in_map_for(gaze, hand, b):
    cg = np.asarray(gaze[b], dtype=np.float32).reshape(NMAPS, 2)
    ch = np.asarray(hand[b], dtype=np.float32).reshape(NMAPS, 2)
    d = np.sqrt(((cg - ch) ** 2).sum(axis=1))
    negc = -np.concatenate([cg, ch], axis=0)  # t-major rows
    negd = -np.concatenate([d, d])[:, None]
    return {"negcd": np.ascontiguousarray(
        np.concatenate([negc, negd], axis=1).astype(np.float32))}


def kernel(gaze_coords, hand_coords, _trace=False, **trace_kwargs):
    gaze_coords = np.asarray(gaze_coords, dtype=np.float32)
    hand_coords = np.asarray(hand_coords, dtype=np.float32)
    B = gaze_coords.shape[0]
    assert B == N_CORES, f"expected batch {N_CORES}, got {B}"
    nc = _build()
    in_maps = [_in_map_for(gaze_coords, hand_coords, b) for b in range(B)]
    res = run_bass_kernel_spmd(nc, in_maps, list(range(N_CORES)),
                               trace=_trace, **trace_kwargs)
    out = np.stack(
        [np.asarray(res.results[i]["out"]).astype(np.float32).reshape(
            S_DIM, C_DIM, H, W) for i in range(B)],
        axis=0,
    )
    if _trace:
        return out, res
    return out
